# revision 3
# baseline (speedup 1.0000x reference)
"""Trainium2 Bass kernel v2 for the dense transformer block (B=4,T=2048,D=1024,H=16).

Same sharding as v1 (4 pairs x 2 ranks, head-split attention + 2-rank RS,
token-split MLP). Restructured for overlap:
  - LN gains/biases folded into weights host-side; LN = bn_stats + one ACT op.
  - A/B interleaved per token chunk; V matmuls before QK; attention chunk qc
    emitted right after its QK chunk.
  - Attention inner loop: score-MM pairs into 2-bank PSUM, ONE exp per
    [128,1024] pair, skewed one pair ahead of the AV matmuls.
  - Softmax denominators batched per chunk: one [8,512] DVE reciprocal,
    broadcast to head-pairs via a select-matrix matmul.
  - Post-chains (wo partial + RS + residual/LN2) emitted early, inside the
    next attention chunk.
  - MLP FC1 units interleaved into the ACT-bound attention phase; FC2 uses
    8 PSUM accumulators with wproj streamed (no 8MB resident tile).
"""

import sys
from contextlib import ExitStack

sys.path.insert(0, "/opt/trn_rl_repo")

import numpy as np
import ml_dtypes

import concourse.bass as bass
import concourse.tile as tile
from concourse import mybir
from concourse.bass_utils import run_bass_kernel_spmd
from concourse.masks import make_identity

BF16 = mybir.dt.bfloat16
F32 = mybir.dt.float32
AF = mybir.ActivationFunctionType

T = 2048
D = 1024
H = 16
HD = 64
HL = 8
P = 128
NT = T // P  # 16
ND = D // P  # 8
TL = T // 2  # 1024
NTL = TL // P  # 8
FC = 4 * D
NFC = FC // P  # 32
EPS = 1e-5


def _patch_tile_drain():
    from concourse.tile import ScopedClock

    def patched(self, tick_clock, wait_clock):
        nc = self.nc
        probe = nc.sync.nop(nofuse=True)
        wait_clock.add_sem_waits(probe.ins, ScopedClock({None: tick_clock.global_clock}))
        si = probe.ins.sync_info
        waits = list(si.on_wait) if si and si.on_wait else []
        if len(waits) > 1:
            probe.ins.sync_info = mybir.SyncInfo(
                on_wait=waits[:1], on_update=list(si.on_update or [])
            )
            for i in range(1, len(waits)):
                nop = nc.sync.nop(nofuse=True)
                nop.ins.sync_info = mybir.SyncInfo(on_wait=waits[i : i + 1], on_update=[])
        nc.all_engine_barrier()
        popped = nc._tile_sem_poison_stack.pop()
        assert popped is self._sem_poison
        nc.clear_and_free_semaphores(list(self.sems.allocated().values()))
        nc.all_engine_barrier()

    tile.TileContext._drain_and_barrier = patched


_SYNC_WAIT_CAP = 1


def _split_sync_waits(nc, cap=_SYNC_WAIT_CAP):
    cnt = 0
    for f in nc.m.functions:
        for bb in f.blocks:
            new_insts = []
            for inst in bb.instructions:
                si = inst.sync_info
                waits = list(si.on_wait) if si and si.on_wait else []
                if len(waits) > cap:
                    for i in range(0, len(waits) - cap, cap):
                        nop = mybir.InstNoOp(name=f"waitsplit_{cnt}", ins=[], outs=[])
                        cnt += 1
                        nop.engine = inst.engine
                        nop.sync_info = mybir.SyncInfo(
                            on_wait=waits[i : i + cap], on_update=[]
                        )
                        new_insts.append(nop)
                    inst.sync_info = mybir.SyncInfo(
                        on_wait=waits[len(waits) - cap :],
                        on_update=list(si.on_update or []),
                    )
                new_insts.append(inst)
            bb.instructions[:] = new_insts


def _bcast_ap(ap_1d, parts):
    return bass.AP(
        tensor=ap_1d.tensor,
        offset=ap_1d.offset,
        ap=[[0, parts], list(ap_1d.ap[0])],
    )


def build_kernel():
    nc = bass.Bass()

    x_ext = nc.declare_dram_parameter("x", [T, D], F32, isOutput=False)
    xres_ext = nc.declare_dram_parameter("x_res", [TL, D], F32, isOutput=False)
    wqk_ext = nc.declare_dram_parameter("wqk", [D, D], BF16, isOutput=False)
    wv_ext = nc.declare_dram_parameter("wv", [D, HL * HD], BF16, isOutput=False)
    wo_ext = nc.declare_dram_parameter("wo", [HL * HD, D], BF16, isOutput=False)
    wfc_ext = nc.declare_dram_parameter("wfc", [D, FC], BF16, isOutput=False)
    wproj_ext = nc.declare_dram_parameter("wproj", [FC, D], BF16, isOutput=False)
    bfc_ext = nc.declare_dram_parameter("bfc", [FC], F32, isOutput=False)
    bproj_ext = nc.declare_dram_parameter("bproj", [D], F32, isOutput=False)
    qkb_ext = nc.declare_dram_parameter("qkb", [D], F32, isOutput=False)
    masks2_ext = nc.declare_dram_parameter("masks2", [4, P, 512], BF16, isOutput=False)
    sel_ext = nc.declare_dram_parameter("sel", [HL, 512], BF16, isOutput=False)
    out_ext = nc.declare_dram_parameter("out", [TL, D], F32, isOutput=True)

    cc_ins = [nc.dram_tensor(f"cc_in{c}", [512, D], BF16) for c in range(4)]
    cc_outs = [nc.dram_tensor(f"cc_out{c}", [256, D], BF16) for c in range(4)]
    resid_dram = nc.dram_tensor("resid_dram", [TL, D], F32)

    x_r = x_ext.rearrange("(t p) d -> p t d", p=P)
    xres_r = xres_ext.rearrange("(t p) d -> p t d", p=P)
    wqk_r = wqk_ext.rearrange("(dt p) c -> p dt c", p=P)
    wv_r = wv_ext.rearrange("(dt p) c -> p dt c", p=P)
    wo_r = wo_ext.rearrange("(yt p) c -> p yt c", p=P)
    wfc_r = wfc_ext.rearrange("(dt p) c -> p dt c", p=P)
    wproj_r = wproj_ext.rearrange("(f p) d -> p f d", p=P)
    masks2_r = masks2_ext.rearrange("j p q -> p j q")
    qkb_r = qkb_ext.rearrange("(t p) -> p t", p=P)
    cc_in_rs = [t.rearrange("(t p) d -> p t d", p=P) for t in cc_ins]
    cc_out_rs = [t.rearrange("(t p) d -> p t d", p=P) for t in cc_outs]
    resid_r = resid_dram.rearrange("(t p) d -> p t d", p=P)
    out_r = out_ext.rearrange("(t p) d -> p t d", p=P)

    with tile.TileContext(nc) as tc, ExitStack() as es:
        singles = es.enter_context(tc.tile_pool(name="singles", bufs=1))
        pBC = es.enter_context(tc.tile_pool(name="pBC", bufs=1))
        sc = es.enter_context(tc.tile_pool(name="sc", bufs=3))
        scE = es.enter_context(tc.tile_pool(name="scE", bufs=2))

        # ---- constants ----
        ident = singles.tile([P, P], BF16)
        make_identity(nc, ident)
        bproj_sb = singles.tile([P, D], F32)
        nc.scalar.dma_start(out=bproj_sb[:], in_=_bcast_ap(bproj_ext[:], P))
        bfc_sb = singles.tile([P, NFC], F32)
        nc.scalar.dma_start(out=bfc_sb[:], in_=bfc_ext.rearrange("(o p) -> p o", p=P))
        masks2_sb = singles.tile([P, 4, 512], BF16)
        nc.scalar.dma_start(out=masks2_sb[:], in_=masks2_r[:])
        qkb_sb = singles.tile([P, ND], F32)
        nc.scalar.dma_start(out=qkb_sb[:], in_=qkb_r[:])
        sel_sb = singles.tile([HL, 4, P], BF16)
        nc.scalar.dma_start(out=sel_sb[:], in_=sel_ext.rearrange("k (hp m) -> k hp m", m=P))
        hT = singles.tile([P, ND, TL], BF16)  # LN2(resid)^T

        # ---- persistent SBUF for attention ----
        qkT = pBC.tile([P, ND, T], BF16)  # [qcols|kcols, tok]
        v_sb = pBC.tile([P, NT, HL, HD + 1], BF16)
        nc.vector.memset(v_sb[:, :, :, HD : HD + 1], 1.0)
        yT = pBC.tile([P, HL * HD // P, T], BF16)  # [ydim, tok]
        wo_sb = pBC.tile([P, HL * HD // P, D], BF16)
        nc.scalar.dma_start(out=wo_sb[:], in_=wo_r[:])

        def layernorm_fold(pool, x_tile, out_bf):
            """out_bf = (x - mu) * rstd  (gain/bias folded into weights)."""
            stats = pool.tile([P, 2, 6], F32, tag="ln_stats")
            for s in range(2):
                nc.vector.bn_stats(
                    out=stats[:, s, :], in_=x_tile[:, s * 512 : (s + 1) * 512]
                )
            mv = pool.tile([P, 2], F32, tag="ln_mv")
            nc.vector.bn_aggr(out=mv[:], in_=stats[:])
            # rstd = 1/sqrt(var+eps) via Newton on DVE (keeps ACT tables
            # free for Exp/Gelu; LN input variance is ~1 so 3 steps converge)
            ve = pool.tile([P, 1], F32, tag="ln_ve")
            nc.vector.tensor_scalar(
                out=ve[:], in0=mv[:, 1:2], scalar1=EPS, scalar2=None,
                op0=mybir.AluOpType.add,
            )
            y = pool.tile([P, 1], F32, tag="ln_y")
            nc.vector.tensor_scalar(
                out=y[:], in0=ve[:], scalar1=-0.5, scalar2=1.5,
                op0=mybir.AluOpType.mult, op1=mybir.AluOpType.add,
            )
            w = pool.tile([P, 1], F32, tag="ln_w")
            for _ in range(2):
                nc.vector.tensor_mul(out=w[:], in0=y[:], in1=y[:])
                nc.vector.tensor_scalar(
                    out=w[:], in0=w[:], scalar1=ve[:], scalar2=-0.5,
                    op0=mybir.AluOpType.mult, op1=mybir.AluOpType.mult,
                )
                nc.vector.tensor_scalar(
                    out=w[:], in0=w[:], scalar1=1.5, scalar2=None,
                    op0=mybir.AluOpType.add,
                )
                nc.vector.tensor_mul(out=y[:], in0=y[:], in1=w[:])
            nmr = pool.tile([P, 1], F32, tag="ln_nmr")
            nc.vector.tensor_scalar(
                out=nmr[:],
                in0=mv[:, 0:1],
                scalar1=y[:],
                scalar2=-1.0,
                op0=mybir.AluOpType.mult,
                op1=mybir.AluOpType.mult,
            )
            nc.scalar.activation(
                out=out_bf[:], in_=x_tile[:], func=AF.Identity,
                bias=nmr[:], scale=y[:],
            )

        psMM = es.enter_context(tc.tile_pool(name="psMM", bufs=2, space="PSUM"))
        psS = es.enter_context(tc.tile_pool(name="psS", bufs=2, space="PSUM"))
        psY = es.enter_context(tc.tile_pool(name="psY", bufs=2, space="PSUM"))

        # ========== stage E (inside post_chain) ==========
        def stage_e(pqc):
            for i2 in range(2):
                tt = 2 * pqc + i2
                rs_bf = scE.tile([P, D], BF16, tag="rs_bf")
                nc.gpsimd.dma_start(out=rs_bf[:], in_=cc_out_rs[pqc][:, i2, :])
                xr = scE.tile([P, D], F32, tag="xr")
                nc.gpsimd.dma_start(out=xr[:], in_=xres_r[:, tt, :])
                nc.vector.tensor_add(out=xr[:], in0=xr[:], in1=rs_bf[:])
                nc.gpsimd.dma_start(out=resid_r[:, tt, :], in_=xr[:])
                h2 = sc.tile([P, D], BF16, tag="xl")
                layernorm_fold(scE, xr, h2)
                for dt in range(ND):
                    tp2 = psMM.tile([P, P], BF16, tag="mm")
                    nc.tensor.transpose(tp2[:], h2[:, dt * P : (dt + 1) * P], ident[:])
                    nc.vector.tensor_copy(
                        out=hT[:, dt, tt * P : (tt + 1) * P], in_=tp2[:]
                    )

        # ========== post chain: D (wo partial) + RS ==========
        def post_d(pqc):
            for t4 in range(4):
                tt = 4 * pqc + t4
                for half in range(2):
                    op = psMM.tile([P, 512], F32, tag="mm")
                    for yt in range(HL * HD // P):
                        nc.tensor.matmul(
                            op[:],
                            lhsT=yT[:, yt, tt * P : (tt + 1) * P],
                            rhs=wo_sb[:, yt, half * 512 : (half + 1) * 512],
                            start=(yt == 0),
                            stop=(yt == HL * HD // P - 1),
                        )
                    ob = sc.tile([P, 512], BF16, tag="ob", bufs=2)
                    nc.vector.tensor_copy(out=ob[:], in_=op[:])
                    nc.gpsimd.dma_start(
                        out=cc_in_rs[pqc][:, t4, half * 512 : (half + 1) * 512],
                        in_=ob[:],
                    )
            nc.gpsimd.collective_compute(
                "ReduceScatter",
                mybir.AluOpType.add,
                ins=[cc_ins[pqc][:]],
                outs=[cc_outs[pqc][:]],
                replica_groups=[[0, 1], [2, 3], [4, 5], [6, 7]],
            )

        # ========== attention chunk ==========
        norm_state = {"pending": None}

        def emit_chunk_norm():
            if norm_state["pending"] is None:
                return
            qc, ys2_list, dsum8 = norm_state["pending"]
            norm_state["pending"] = None
            nc.vector.reciprocal(out=dsum8[:], in_=dsum8[:])
            recb8 = sc.tile([HL, 512], BF16, tag="recb8", bufs=2)
            nc.vector.tensor_copy(out=recb8[:], in_=dsum8[:])
            for hp in range(4):
                bp2 = psMM.tile([P, 512], F32, tag="mm")
                nc.tensor.matmul(
                    bp2[:], lhsT=sel_sb[:, hp, :], rhs=recb8[:], start=True, stop=True
                )
                bps2 = sc.tile([P, 512], BF16, tag="bps2", bufs=2)
                nc.vector.tensor_copy(out=bps2[:], in_=bp2[:])
                nc.vector.tensor_mul(
                    out=yT[:, hp, qc * 512 : (qc + 1) * 512],
                    in0=ys2_list[hp][:],
                    in1=bps2[:],
                )

        def attn(qc, after_head=None):
            nkt = 4 * (qc + 1)
            dsum8 = sc.tile([HL, 512], F32, tag="dsum8", bufs=2, name=f"dsum8_q{qc}")
            ys2_list = []
            for hp in range(4):
                hA, hB = 2 * hp, 2 * hp + 1
                qt_ct = hp
                kt_ct = 4 + hp
                ys2 = sc.tile([P, 512], BF16, tag="ys2", bufs=6, name=f"ys2_q{qc}p{hp}")
                ys2_list.append(ys2)
                ypA = psY.tile([HD + 1, 512], F32, tag="yp", name=f"ypA_q{qc}p{hp}")
                ypB = psY.tile([HD + 1, 512], F32, tag="yp", name=f"ypB_q{qc}p{hp}")
                pending_av = [None]

                def emit_av(ypA=ypA, ypB=ypB, pending_av=pending_av, nkt=nkt, hA=hA, hB=hB):
                    if pending_av[0] is None:
                        return
                    pt2, kt = pending_av[0]
                    pending_av[0] = None
                    nc.tensor.matmul(
                        ypA[:], lhsT=v_sb[:, kt, hA, :], rhs=pt2[:, 0:512],
                        start=(kt == 0), stop=(kt == nkt - 1),
                    )
                    nc.tensor.matmul(
                        ypB[:], lhsT=v_sb[:, kt, hB, :], rhs=pt2[:, 512:1024],
                        start=(kt == 0), stop=(kt == nkt - 1),
                    )

                for kt in range(nkt):
                    ps2 = psS.tile([P, 1024], F32, tag="s2", name=f"s_q{qc}p{hp}k{kt}")
                    # two heads' score MMs run concurrently in disjoint row
                    # strips (K=64 each, bases 0 and 64) into separate banks
                    nc.tensor.matmul(
                        ps2[:, 0:512],
                        lhsT=qkT[0:HD, kt_ct, kt * P : (kt + 1) * P],
                        rhs=qkT[0:HD, qt_ct, qc * 512 : (qc + 1) * 512],
                        start=True, stop=True,
                    )
                    nc.tensor.matmul(
                        ps2[:, 512:1024],
                        lhsT=qkT[HD:P, kt_ct, kt * P : (kt + 1) * P],
                        rhs=qkT[HD:P, qt_ct, qc * 512 : (qc + 1) * 512],
                        start=True, stop=True,
                    )
                    if kt == 0 and hp == 0:
                        emit_chunk_norm()  # previous chunk's normalize
                    pt2 = sc.tile([P, 1024], BF16, tag="pt2", name=f"pt_q{qc}p{hp}k{kt}")
                    nc.scalar.activation(
                        out=pt2[:], in_=ps2[:], func=AF.Exp, scale=0.125
                    )
                    j = kt - 4 * qc
                    if j >= 0:
                        for hh in range(2):
                            nc.vector.tensor_mul(
                                out=pt2[:, hh * 512 : (hh + 1) * 512],
                                in0=pt2[:, hh * 512 : (hh + 1) * 512],
                                in1=masks2_sb[:, j, :],
                            )
                    emit_av()
                    pending_av[0] = (pt2, kt)
                emit_av()
                nc.vector.tensor_copy(out=ys2[0:HD, :], in_=ypA[0:HD, :])
                nc.vector.tensor_copy(out=ys2[HD:P, :], in_=ypB[0:HD, :])
                for h, yp in ((hA, ypA), (hB, ypB)):
                    dcp = sc.tile([1, 512], F32, tag="dcp", name=f"dcp_q{qc}h{h}")
                    nc.vector.tensor_copy(out=dcp[:], in_=yp[HD : HD + 1, :])
                    nc.gpsimd.dma_start(out=dsum8[h : h + 1, :], in_=dcp[:])
                if after_head is not None:
                    for cb in after_head.get(hp, []):
                        cb()
            norm_state["pending"] = (qc, ys2_list, dsum8)

        # ========== MLP pieces ==========
        pF_state = {"fc1tag": "s2"}

        def fc1_unit(tc2, fct, psum_pool, sfpool):
            wfc_tile = sfpool.tile([P, ND, P], BF16, tag="wfc_t", bufs=4)
            nc.gpsimd.dma_start(
                out=wfc_tile[:], in_=wfc_r[:, :, fct * P : (fct + 1) * P]
            )
            fp = psum_pool.tile([P, 512], F32, tag=pF_state["fc1tag"])
            for dt in range(ND):
                nc.tensor.matmul(
                    fp[:],
                    lhsT=wfc_tile[:, dt, :],
                    rhs=hT[:, dt, tc2 * 512 : (tc2 + 1) * 512],
                    start=(dt == 0),
                    stop=(dt == ND - 1),
                )
            gT = pF_state["gT"]
            nc.scalar.activation(
                out=gT[:, fct, :],
                in_=fp[:],
                func=AF.Gelu_apprx_tanh,
                bias=bfc_sb[:, fct : fct + 1],
                scale=1.0,
            )

        def fc2_block(tc2, t4pair):
            gT = pF_state["gT"]
            accs = [
                psS.tile([P, 1024], F32, tag="s2", name=f"facc{tc2}_{t4pair}_{i}")
                for i in range(2)
            ]
            for fct in range(NFC):
                wp = sfw.tile(
                    [P, D], BF16, tag="wp_t", bufs=4, name=f"wp{tc2}_{t4pair}_{fct}"
                )
                nc.sync.dma_start(out=wp[:], in_=wproj_r[:, fct, :])
                for i2 in range(2):
                    t4 = 2 * t4pair + i2
                    for half in range(2):
                        nc.tensor.matmul(
                            accs[i2][:, half * 512 : (half + 1) * 512],
                            lhsT=gT[:, fct, t4 * P : (t4 + 1) * P],
                            rhs=wp[:, half * 512 : (half + 1) * 512],
                            start=(fct == 0),
                            stop=(fct == NFC - 1),
                        )
            for i2 in range(2):
                t4 = 2 * t4pair + i2
                tt = tc2 * 4 + t4
                for half in range(2):
                    rt = sfout.tile([P, 512], F32, tag="rt")
                    nc.sync.dma_start(
                        out=rt[:], in_=resid_r[:, tt, half * 512 : (half + 1) * 512]
                    )
                    nc.vector.tensor_add(
                        out=rt[:],
                        in0=accs[i2][:, half * 512 : (half + 1) * 512],
                        in1=rt[:],
                    )
                    nc.vector.tensor_add(
                        out=rt[:],
                        in0=rt[:],
                        in1=bproj_sb[:, half * 512 : (half + 1) * 512],
                    )
                    nc.sync.dma_start(
                        out=out_r[:, tt, half * 512 : (half + 1) * 512], in_=rt[:]
                    )

        # ========== A + B interleaved with attention ==========
        with tc.tile_pool(name="pAB", bufs=1) as pAB:
            wqk_sb = pAB.tile([P, ND, D], BF16)
            nc.scalar.dma_start(out=wqk_sb[:], in_=wqk_r[:])
            wv_sb = pAB.tile([P, ND, HL * HD], BF16)
            nc.scalar.dma_start(out=wv_sb[:], in_=wv_r[:])

            def ab_chunk(qc):
                xlT = pAB.tile([P, ND, 512], BF16, tag="xlT", bufs=2)
                for t4 in range(4):
                    tt = 4 * qc + t4
                    x_tile = sc.tile([P, D], F32, tag="x_tile", bufs=2)
                    nc.sync.dma_start(out=x_tile[:], in_=x_r[:, tt, :])
                    xl = sc.tile([P, D], BF16, tag="xl")
                    layernorm_fold(sc, x_tile, xl)
                    for dt in range(ND):
                        tp = psMM.tile([P, P], BF16, tag="mm")
                        nc.tensor.transpose(
                            tp[:], xl[:, dt * P : (dt + 1) * P], ident[:]
                        )
                        nc.scalar.copy(
                            out=xlT[:, dt, t4 * P : (t4 + 1) * P], in_=tp[:]
                        )
                    vp = psMM.tile([P, 512], F32, tag="mm")
                    for dt in range(ND):
                        nc.tensor.matmul(
                            vp[:],
                            lhsT=xlT[:, dt, t4 * P : (t4 + 1) * P],
                            rhs=wv_sb[:, dt, :],
                            start=(dt == 0),
                            stop=(dt == ND - 1),
                        )
                    nc.vector.tensor_copy(
                        out=v_sb[:, tt, :, 0:HD],
                        in_=vp.rearrange("p (h e) -> p h e", h=HL),
                    )
                for ct in range(ND):
                    qp = psMM.tile([P, 512], F32, tag="mm")
                    for dt in range(ND):
                        nc.tensor.matmul(
                            qp[:],
                            lhsT=wqk_sb[:, dt, ct * P : (ct + 1) * P],
                            rhs=xlT[:, dt, :],
                            start=(dt == 0),
                            stop=(dt == ND - 1),
                        )
                    nc.scalar.activation(
                        out=qkT[:, ct, qc * 512 : (qc + 1) * 512],
                        in_=qp[:],
                        func=AF.Identity,
                        bias=qkb_sb[:, ct : ct + 1],
                        scale=1.0,
                    )

            ab_chunk(0)
            ab_chunk(1)
            attn(0)
            ab_chunk(2)
            attn(1, after_head={0: [lambda: post_d(0)]})
            ab_chunk(3)
            attn(2, after_head={0: [lambda: stage_e(0), lambda: post_d(1)]})

        # ---- F-phase SBUF on the RIGHT side (outlives attention PSUM pools) ----
        pG = es.enter_context(tc.tile_pool(name="pG", bufs=1, side="right"))
        sfw = es.enter_context(tc.tile_pool(name="sfw", bufs=4, side="right"))
        sfout = es.enter_context(tc.tile_pool(name="sfout", bufs=3, side="right"))
        gT = pG.tile([P, NFC, 512], BF16, tag="gT")
        pF_state["gT"] = gT

        units = [(0, fct) for fct in range(NFC)]
        uidx = [0]

        def emit_units(n):
            def cb():
                for _ in range(n):
                    if uidx[0] < len(units):
                        tc2, fct = units[uidx[0]]
                        uidx[0] += 1
                        fc1_unit(tc2, fct, psS, sfw)
            return cb

        sched = {
            0: [lambda: stage_e(1), lambda: post_d(2)],
            2: [lambda: stage_e(2)],
        }
        for hp, n in zip(range(1, 4), (11, 11, 10)):
            sched.setdefault(hp, []).append(emit_units(n))
        attn(3, after_head=sched)
        while uidx[0] < len(units):
            tc2, fct = units[uidx[0]]
            uidx[0] += 1
            fc1_unit(0, fct, psS, sfw)
        fc2_block(0, 0)
        emit_chunk_norm()
        post_d(3)
        fc2_block(0, 1)
        stage_e(3)
        for fct in range(NFC):
            fc1_unit(1, fct, psS, sfw)
        fc2_block(1, 0)
        fc2_block(1, 1)

    _split_sync_waits(nc)
    return nc


_NC_CACHE = None


def _get_nc():
    global _NC_CACHE
    if _NC_CACHE is None:
        _patch_tile_drain()
        _NC_CACHE = build_kernel()
    return _NC_CACHE


def make_in_maps(x, w_attn, w_o, ln1_g, ln1_b, ln2_g, ln2_b, w_fc, b_fc, w_proj, b_proj):
    bf = ml_dtypes.bfloat16
    f32 = np.float32
    x = np.asarray(x, f32)
    w_attn = np.asarray(w_attn, f32)
    w_o = np.asarray(w_o, f32)
    ln1_g = np.asarray(ln1_g, f32)
    ln1_b = np.asarray(ln1_b, f32)
    ln2_g = np.asarray(ln2_g, f32)
    ln2_b = np.asarray(ln2_b, f32)
    w_fc = np.asarray(w_fc, f32)
    b_fc = np.asarray(b_fc, f32)
    w_proj = np.asarray(w_proj, f32)
    b_proj = np.asarray(b_proj, f32)

    q_idx = np.arange(512)[None, :]
    k_idx = np.arange(P)[:, None]
    masks = np.stack(
        [(q_idx >= k_idx + P * j).astype(np.float32) for j in range(4)]
    )  # [4, P, 512]
    masks2 = masks.astype(bf)

    # sel[k, hp*128+m] = 1 iff head index (2hp + m//64) == k — broadcasts the
    # per-head reciprocal rows [8,512] to [128,512] per head-pair via matmul
    sel = np.zeros((HL, 512), np.float32)
    for hp in range(4):
        for m in range(P):
            sel[2 * hp + m // HD, hp * P + m] = 1.0
    sel = sel.astype(bf)

    wq = w_attn[:, :D]
    wk = w_attn[:, D : 2 * D]
    wv = w_attn[:, 2 * D :]

    wq_g = wq * ln1_g[:, None]
    wk_g = wk * ln1_g[:, None]
    wv_g = wv * ln1_g[:, None]
    wfc_g = w_fc * ln2_g[:, None]
    bfc_f = b_fc + ln2_b @ w_fc
    c0 = (ln1_b @ wv) @ w_o  # [D]

    in_maps = []
    for core in range(8):
        p, r = core // 2, core % 2
        hs = r * HL * HD
        wqk = np.concatenate([wq_g[:, hs : hs + 512], wk_g[:, hs : hs + 512]], axis=1)
        qkb = np.concatenate(
            [ln1_b @ wq[:, hs : hs + 512], ln1_b @ wk[:, hs : hs + 512]]
        )
        x_res = (
            np.concatenate(
                [x[p, 512 * c + 256 * r : 512 * c + 256 * r + 256] for c in range(4)],
                axis=0,
            )
            + c0[None, :]
        )
        in_maps.append(
            {
                "x": np.ascontiguousarray(x[p], f32),
                "x_res": np.ascontiguousarray(x_res, f32),
                "wqk": np.ascontiguousarray(wqk).astype(bf),
                "wv": np.ascontiguousarray(wv_g[:, hs : hs + 512]).astype(bf),
                "wo": np.ascontiguousarray(w_o[hs : hs + 512, :]).astype(bf),
                "wfc": np.ascontiguousarray(wfc_g).astype(bf),
                "wproj": np.ascontiguousarray(w_proj).astype(bf),
                "bfc": np.ascontiguousarray(bfc_f, f32),
                "bproj": np.ascontiguousarray(b_proj, f32),
                "qkb": np.ascontiguousarray(qkb, f32),
                "masks2": masks2,
                "sel": sel,
            }
        )
    return in_maps


def kernel(**inputs):
    inputs = {k: np.asarray(v) for k, v in inputs.items()}
    nc = _get_nc()
    in_maps = make_in_maps(**inputs)
    res = run_bass_kernel_spmd(nc, in_maps, core_ids=list(range(8)))
    x = inputs["x"]
    B = x.shape[0]
    out = np.empty((B, T, D), np.float32)
    for core in range(8):
        p, r = core // 2, core % 2
        o = res.results[core]["out"]
        for c in range(4):
            out[p, 512 * c + 256 * r : 512 * c + 256 * r + 256] = o[
                c * 256 : (c + 1) * 256
            ]
    return out


if __name__ == "__main__":
    print("building...")
    nc = _get_nc()
    n = sum(len(bb.instructions) for f in nc.m.functions for bb in f.blocks)
    print("built:", n, "instructions")


# revision 4
# speedup vs baseline: 1.0278x; 1.0278x over previous
"""Trainium2 Bass kernel v2 for the dense transformer block (B=4,T=2048,D=1024,H=16).

Same sharding as v1 (4 pairs x 2 ranks, head-split attention + 2-rank RS,
token-split MLP). Restructured for overlap:
  - LN gains/biases folded into weights host-side; LN = bn_stats + one ACT op.
  - A/B interleaved per token chunk; V matmuls before QK; attention chunk qc
    emitted right after its QK chunk.
  - Attention inner loop: score-MM pairs into 2-bank PSUM, ONE exp per
    [128,1024] pair, skewed one pair ahead of the AV matmuls.
  - Softmax denominators batched per chunk: one [8,512] DVE reciprocal,
    broadcast to head-pairs via a select-matrix matmul.
  - Post-chains (wo partial + RS + residual/LN2) emitted early, inside the
    next attention chunk.
  - MLP FC1 units interleaved into the ACT-bound attention phase; FC2 uses
    8 PSUM accumulators with wproj streamed (no 8MB resident tile).
"""

import sys
from contextlib import ExitStack

sys.path.insert(0, "/opt/trn_rl_repo")

import numpy as np
import ml_dtypes

import concourse.bass as bass
import concourse.tile as tile
from concourse import mybir
from concourse.bass_utils import run_bass_kernel_spmd
from concourse.masks import make_identity

BF16 = mybir.dt.bfloat16
F32 = mybir.dt.float32
AF = mybir.ActivationFunctionType

T = 2048
D = 1024
H = 16
HD = 64
HL = 8
P = 128
NT = T // P  # 16
ND = D // P  # 8
TL = T // 2  # 1024
NTL = TL // P  # 8
FC = 4 * D
NFC = FC // P  # 32
EPS = 1e-5


def _patch_tile_drain():
    from concourse.tile import ScopedClock

    def patched(self, tick_clock, wait_clock):
        nc = self.nc
        probe = nc.sync.nop(nofuse=True)
        wait_clock.add_sem_waits(probe.ins, ScopedClock({None: tick_clock.global_clock}))
        si = probe.ins.sync_info
        waits = list(si.on_wait) if si and si.on_wait else []
        if len(waits) > 1:
            probe.ins.sync_info = mybir.SyncInfo(
                on_wait=waits[:1], on_update=list(si.on_update or [])
            )
            for i in range(1, len(waits)):
                nop = nc.sync.nop(nofuse=True)
                nop.ins.sync_info = mybir.SyncInfo(on_wait=waits[i : i + 1], on_update=[])
        nc.all_engine_barrier()
        popped = nc._tile_sem_poison_stack.pop()
        assert popped is self._sem_poison
        nc.clear_and_free_semaphores(list(self.sems.allocated().values()))
        nc.all_engine_barrier()

    tile.TileContext._drain_and_barrier = patched


_SYNC_WAIT_CAP = 1


def _split_sync_waits(nc, cap=_SYNC_WAIT_CAP):
    cnt = 0
    for f in nc.m.functions:
        for bb in f.blocks:
            new_insts = []
            for inst in bb.instructions:
                si = inst.sync_info
                waits = list(si.on_wait) if si and si.on_wait else []
                if len(waits) > cap:
                    for i in range(0, len(waits) - cap, cap):
                        nop = mybir.InstNoOp(name=f"waitsplit_{cnt}", ins=[], outs=[])
                        cnt += 1
                        nop.engine = inst.engine
                        nop.sync_info = mybir.SyncInfo(
                            on_wait=waits[i : i + cap], on_update=[]
                        )
                        new_insts.append(nop)
                    inst.sync_info = mybir.SyncInfo(
                        on_wait=waits[len(waits) - cap :],
                        on_update=list(si.on_update or []),
                    )
                new_insts.append(inst)
            bb.instructions[:] = new_insts


def _bcast_ap(ap_1d, parts):
    return bass.AP(
        tensor=ap_1d.tensor,
        offset=ap_1d.offset,
        ap=[[0, parts], list(ap_1d.ap[0])],
    )


def build_kernel():
    nc = bass.Bass()

    x_ext = nc.declare_dram_parameter("x", [T, D], F32, isOutput=False)
    xres_ext = nc.declare_dram_parameter("x_res", [TL, D], F32, isOutput=False)
    wqk_ext = nc.declare_dram_parameter("wqk", [D, D], BF16, isOutput=False)
    wv_ext = nc.declare_dram_parameter("wv", [D, HL * HD], BF16, isOutput=False)
    wo_ext = nc.declare_dram_parameter("wo", [HL * HD, D], BF16, isOutput=False)
    wfc_ext = nc.declare_dram_parameter("wfc", [D, FC], BF16, isOutput=False)
    wproj_ext = nc.declare_dram_parameter("wproj", [FC, D], BF16, isOutput=False)
    bfc_ext = nc.declare_dram_parameter("bfc", [FC], F32, isOutput=False)
    bproj_ext = nc.declare_dram_parameter("bproj", [D], F32, isOutput=False)
    qkb_ext = nc.declare_dram_parameter("qkb", [D], F32, isOutput=False)
    masks2_ext = nc.declare_dram_parameter("masks2", [4, P, 512], BF16, isOutput=False)
    sel_ext = nc.declare_dram_parameter("sel", [HL, 512], BF16, isOutput=False)
    out_ext = nc.declare_dram_parameter("out", [TL, D], F32, isOutput=True)

    cc_ins = [nc.dram_tensor(f"cc_in{c}", [512, D], BF16) for c in range(4)]
    cc_outs = [nc.dram_tensor(f"cc_out{c}", [256, D], BF16) for c in range(4)]
    resid_dram = nc.dram_tensor("resid_dram", [TL, D], F32)

    x_r = x_ext.rearrange("(t p) d -> p t d", p=P)
    xres_r = xres_ext.rearrange("(t p) d -> p t d", p=P)
    wqk_r = wqk_ext.rearrange("(dt p) c -> p dt c", p=P)
    wv_r = wv_ext.rearrange("(dt p) c -> p dt c", p=P)
    wo_r = wo_ext.rearrange("(yt p) c -> p yt c", p=P)
    wfc_r = wfc_ext.rearrange("(dt p) c -> p dt c", p=P)
    wproj_r = wproj_ext.rearrange("(f p) d -> p f d", p=P)
    masks2_r = masks2_ext.rearrange("j p q -> p j q")
    qkb_r = qkb_ext.rearrange("(t p) -> p t", p=P)
    cc_in_rs = [t.rearrange("(t p) d -> p t d", p=P) for t in cc_ins]
    cc_out_rs = [t.rearrange("(t p) d -> p t d", p=P) for t in cc_outs]
    resid_r = resid_dram.rearrange("(t p) d -> p t d", p=P)
    out_r = out_ext.rearrange("(t p) d -> p t d", p=P)

    with tile.TileContext(nc) as tc, ExitStack() as es:
        singles = es.enter_context(tc.tile_pool(name="singles", bufs=1))
        pBC = es.enter_context(tc.tile_pool(name="pBC", bufs=1))
        sc = es.enter_context(tc.tile_pool(name="sc", bufs=3))
        scE = es.enter_context(tc.tile_pool(name="scE", bufs=2))

        # ---- constants ----
        ident = singles.tile([P, P], BF16)
        make_identity(nc, ident)
        bproj_sb = singles.tile([P, D], F32)
        nc.scalar.dma_start(out=bproj_sb[:], in_=_bcast_ap(bproj_ext[:], P))
        bfc_sb = singles.tile([P, NFC], F32)
        nc.scalar.dma_start(out=bfc_sb[:], in_=bfc_ext.rearrange("(o p) -> p o", p=P))
        masks2_sb = singles.tile([P, 4, 512], BF16)
        nc.scalar.dma_start(out=masks2_sb[:], in_=masks2_r[:])
        qkb_sb = singles.tile([P, ND], F32)
        nc.scalar.dma_start(out=qkb_sb[:], in_=qkb_r[:])
        sel_sb = singles.tile([HL, 4, P], BF16)
        nc.scalar.dma_start(out=sel_sb[:], in_=sel_ext.rearrange("k (hp m) -> k hp m", m=P))
        hT = singles.tile([P, ND, TL], BF16)  # LN2(resid)^T

        # ---- persistent SBUF for attention ----
        qkT = pBC.tile([P, ND, T], BF16)  # [qcols|kcols, tok]
        v_sb = pBC.tile([P, NT, HL, HD + 1], BF16)
        nc.vector.memset(v_sb[:, :, :, HD : HD + 1], 1.0)
        yT = pBC.tile([P, HL * HD // P, T], BF16)  # [ydim, tok]
        wo_sb = pBC.tile([P, HL * HD // P, D], BF16)
        nc.scalar.dma_start(out=wo_sb[:], in_=wo_r[:])

        def layernorm_fold(pool, x_tile, out_bf):
            """out_bf = (x - mu) * rstd  (gain/bias folded into weights)."""
            stats = pool.tile([P, 2, 6], F32, tag="ln_stats")
            for s in range(2):
                nc.vector.bn_stats(
                    out=stats[:, s, :], in_=x_tile[:, s * 512 : (s + 1) * 512]
                )
            mv = pool.tile([P, 2], F32, tag="ln_mv")
            nc.vector.bn_aggr(out=mv[:], in_=stats[:])
            # rstd = 1/sqrt(var+eps) via Newton on DVE (keeps ACT tables
            # free for Exp/Gelu; LN input variance is ~1 so 3 steps converge)
            ve = pool.tile([P, 1], F32, tag="ln_ve")
            nc.vector.tensor_scalar(
                out=ve[:], in0=mv[:, 1:2], scalar1=EPS, scalar2=None,
                op0=mybir.AluOpType.add,
            )
            y = pool.tile([P, 1], F32, tag="ln_y")
            nc.vector.tensor_scalar(
                out=y[:], in0=ve[:], scalar1=-0.5, scalar2=1.5,
                op0=mybir.AluOpType.mult, op1=mybir.AluOpType.add,
            )
            w = pool.tile([P, 1], F32, tag="ln_w")
            for _ in range(2):
                nc.vector.tensor_mul(out=w[:], in0=y[:], in1=y[:])
                nc.vector.tensor_scalar(
                    out=w[:], in0=w[:], scalar1=ve[:], scalar2=-0.5,
                    op0=mybir.AluOpType.mult, op1=mybir.AluOpType.mult,
                )
                nc.vector.tensor_scalar(
                    out=w[:], in0=w[:], scalar1=1.5, scalar2=None,
                    op0=mybir.AluOpType.add,
                )
                nc.vector.tensor_mul(out=y[:], in0=y[:], in1=w[:])
            nmr = pool.tile([P, 1], F32, tag="ln_nmr")
            nc.vector.tensor_scalar(
                out=nmr[:],
                in0=mv[:, 0:1],
                scalar1=y[:],
                scalar2=-1.0,
                op0=mybir.AluOpType.mult,
                op1=mybir.AluOpType.mult,
            )
            nc.scalar.activation(
                out=out_bf[:], in_=x_tile[:], func=AF.Identity,
                bias=nmr[:], scale=y[:],
            )

        psMM = es.enter_context(tc.tile_pool(name="psMM", bufs=2, space="PSUM"))
        psS = es.enter_context(tc.tile_pool(name="psS", bufs=2, space="PSUM"))
        psY = es.enter_context(tc.tile_pool(name="psY", bufs=2, space="PSUM"))

        # ========== stage E (inside post_chain) ==========
        def stage_e(pqc):
            for i2 in range(2):
                tt = 2 * pqc + i2
                rs_bf = scE.tile([P, D], BF16, tag="rs_bf")
                nc.gpsimd.dma_start(out=rs_bf[:], in_=cc_out_rs[pqc][:, i2, :])
                xr = scE.tile([P, D], F32, tag="xr")
                nc.gpsimd.dma_start(out=xr[:], in_=xres_r[:, tt, :])
                nc.vector.tensor_add(out=xr[:], in0=xr[:], in1=rs_bf[:])
                nc.gpsimd.dma_start(out=resid_r[:, tt, :], in_=xr[:])
                h2 = sc.tile([P, D], BF16, tag="xl")
                layernorm_fold(scE, xr, h2)
                for dt in range(ND):
                    tp2 = psMM.tile([P, P], BF16, tag="mm")
                    nc.tensor.transpose(tp2[:], h2[:, dt * P : (dt + 1) * P], ident[:])
                    nc.vector.tensor_copy(
                        out=hT[:, dt, tt * P : (tt + 1) * P], in_=tp2[:]
                    )

        # ========== post chain: D (wo partial) + RS ==========
        def post_d(pqc):
            for t4 in range(4):
                tt = 4 * pqc + t4
                for half in range(2):
                    op = psMM.tile([P, 512], F32, tag="mm")
                    for yt in range(HL * HD // P):
                        nc.tensor.matmul(
                            op[:],
                            lhsT=yT[:, yt, tt * P : (tt + 1) * P],
                            rhs=wo_sb[:, yt, half * 512 : (half + 1) * 512],
                            start=(yt == 0),
                            stop=(yt == HL * HD // P - 1),
                        )
                    ob = sc.tile([P, 512], BF16, tag="ob", bufs=2)
                    nc.vector.tensor_copy(out=ob[:], in_=op[:])
                    nc.gpsimd.dma_start(
                        out=cc_in_rs[pqc][:, t4, half * 512 : (half + 1) * 512],
                        in_=ob[:],
                    )
            nc.gpsimd.collective_compute(
                "ReduceScatter",
                mybir.AluOpType.add,
                ins=[cc_ins[pqc][:]],
                outs=[cc_outs[pqc][:]],
                replica_groups=[[0, 1], [2, 3], [4, 5], [6, 7]],
            )

        # ========== attention chunk ==========
        norm_state = {"pending": None}

        def emit_chunk_norm():
            if norm_state["pending"] is None:
                return
            qc, ys2_list, dsum8 = norm_state["pending"]
            norm_state["pending"] = None
            nc.vector.reciprocal(out=dsum8[:], in_=dsum8[:])
            recb8 = sc.tile([HL, 512], BF16, tag="recb8", bufs=2)
            nc.vector.tensor_copy(out=recb8[:], in_=dsum8[:])
            for hp in range(4):
                bp2 = psMM.tile([P, 512], F32, tag="mm")
                nc.tensor.matmul(
                    bp2[:], lhsT=sel_sb[:, hp, :], rhs=recb8[:], start=True, stop=True
                )
                bps2 = sc.tile([P, 512], BF16, tag="bps2", bufs=2)
                nc.vector.tensor_copy(out=bps2[:], in_=bp2[:])
                nc.vector.tensor_mul(
                    out=yT[:, hp, qc * 512 : (qc + 1) * 512],
                    in0=ys2_list[hp][:],
                    in1=bps2[:],
                )

        def attn(qc, after_head=None):
            nkt = 4 * (qc + 1)
            dsum8 = sc.tile([HL, 512], F32, tag="dsum8", bufs=2, name=f"dsum8_q{qc}")
            ys2_list = []
            for hp in range(4):
                hA, hB = 2 * hp, 2 * hp + 1
                qt_ct = hp
                kt_ct = 4 + hp
                ys2 = sc.tile([P, 512], BF16, tag="ys2", bufs=6, name=f"ys2_q{qc}p{hp}")
                ys2_list.append(ys2)
                ypA = psY.tile([HD + 1, 512], F32, tag="yp", name=f"ypA_q{qc}p{hp}")
                ypB = psY.tile([HD + 1, 512], F32, tag="yp", name=f"ypB_q{qc}p{hp}")
                pending_av = [None]

                def emit_av(ypA=ypA, ypB=ypB, pending_av=pending_av, nkt=nkt, hA=hA, hB=hB):
                    if pending_av[0] is None:
                        return
                    pt2, kt = pending_av[0]
                    pending_av[0] = None
                    nc.tensor.matmul(
                        ypA[:], lhsT=v_sb[:, kt, hA, :], rhs=pt2[:, 0:512],
                        start=(kt == 0), stop=(kt == nkt - 1),
                    )
                    nc.tensor.matmul(
                        ypB[:], lhsT=v_sb[:, kt, hB, :], rhs=pt2[:, 512:1024],
                        start=(kt == 0), stop=(kt == nkt - 1),
                    )

                for kt in range(nkt):
                    ps2 = psS.tile([P, 1024], F32, tag="s2", name=f"s_q{qc}p{hp}k{kt}")
                    # two heads' score MMs run concurrently in disjoint row
                    # strips (K=64 each, bases 0 and 64) into separate banks
                    nc.tensor.matmul(
                        ps2[:, 0:512],
                        lhsT=qkT[0:HD, kt_ct, kt * P : (kt + 1) * P],
                        rhs=qkT[0:HD, qt_ct, qc * 512 : (qc + 1) * 512],
                        start=True, stop=True,
                    )
                    nc.tensor.matmul(
                        ps2[:, 512:1024],
                        lhsT=qkT[HD:P, kt_ct, kt * P : (kt + 1) * P],
                        rhs=qkT[HD:P, qt_ct, qc * 512 : (qc + 1) * 512],
                        start=True, stop=True,
                    )
                    if kt == 0 and hp == 0:
                        emit_chunk_norm()  # previous chunk's normalize
                    pt2 = sc.tile([P, 1024], BF16, tag="pt2", name=f"pt_q{qc}p{hp}k{kt}")
                    nc.scalar.activation(
                        out=pt2[:], in_=ps2[:], func=AF.Exp, scale=0.125
                    )
                    j = kt - 4 * qc
                    if j >= 0:
                        for hh in range(2):
                            nc.vector.tensor_mul(
                                out=pt2[:, hh * 512 : (hh + 1) * 512],
                                in0=pt2[:, hh * 512 : (hh + 1) * 512],
                                in1=masks2_sb[:, j, :],
                            )
                    emit_av()
                    pending_av[0] = (pt2, kt)
                emit_av()
                nc.vector.tensor_copy(out=ys2[0:HD, :], in_=ypA[0:HD, :])
                nc.vector.tensor_copy(out=ys2[HD:P, :], in_=ypB[0:HD, :])
                for h, yp in ((hA, ypA), (hB, ypB)):
                    dcp = sc.tile([1, 512], F32, tag="dcp", name=f"dcp_q{qc}h{h}")
                    nc.vector.tensor_copy(out=dcp[:], in_=yp[HD : HD + 1, :])
                    nc.gpsimd.dma_start(out=dsum8[h : h + 1, :], in_=dcp[:])
                if after_head is not None:
                    for cb in after_head.get(hp, []):
                        cb()
            norm_state["pending"] = (qc, ys2_list, dsum8)

        # ========== MLP pieces ==========
        pF_state = {"fc1tag": "s2"}

        def fc1_unit(tc2, fct, psum_pool, sfpool):
            wfc_tile = sfpool.tile([P, ND, P], BF16, tag="wfc_t", bufs=4)
            nc.sync.dma_start(
                out=wfc_tile[:], in_=wfc_r[:, :, fct * P : (fct + 1) * P]
            )
            fp = psum_pool.tile([P, 512], F32, tag=pF_state["fc1tag"])
            for dt in range(ND):
                nc.tensor.matmul(
                    fp[:],
                    lhsT=wfc_tile[:, dt, :],
                    rhs=hT[:, dt, tc2 * 512 : (tc2 + 1) * 512],
                    start=(dt == 0),
                    stop=(dt == ND - 1),
                )
            gT = pF_state["gT"]
            nc.scalar.activation(
                out=gT[:, fct, :],
                in_=fp[:],
                func=AF.Gelu_apprx_tanh,
                bias=bfc_sb[:, fct : fct + 1],
                scale=1.0,
            )

        def fc2_block(tc2, t4pair):
            gT = pF_state["gT"]
            accs = [
                psS.tile([P, 1024], F32, tag="s2", name=f"facc{tc2}_{t4pair}_{i}")
                for i in range(2)
            ]
            for fct in range(NFC):
                wp = sfw.tile(
                    [P, D], BF16, tag="wp_t", bufs=4, name=f"wp{tc2}_{t4pair}_{fct}"
                )
                nc.sync.dma_start(out=wp[:], in_=wproj_r[:, fct, :])
                for i2 in range(2):
                    t4 = 2 * t4pair + i2
                    for half in range(2):
                        nc.tensor.matmul(
                            accs[i2][:, half * 512 : (half + 1) * 512],
                            lhsT=gT[:, fct, t4 * P : (t4 + 1) * P],
                            rhs=wp[:, half * 512 : (half + 1) * 512],
                            start=(fct == 0),
                            stop=(fct == NFC - 1),
                        )
            for i2 in range(2):
                t4 = 2 * t4pair + i2
                tt = tc2 * 4 + t4
                for half in range(2):
                    rt = sfout.tile([P, 512], F32, tag="rt")
                    nc.sync.dma_start(
                        out=rt[:], in_=resid_r[:, tt, half * 512 : (half + 1) * 512]
                    )
                    nc.vector.tensor_add(
                        out=rt[:],
                        in0=accs[i2][:, half * 512 : (half + 1) * 512],
                        in1=rt[:],
                    )
                    nc.vector.tensor_add(
                        out=rt[:],
                        in0=rt[:],
                        in1=bproj_sb[:, half * 512 : (half + 1) * 512],
                    )
                    nc.sync.dma_start(
                        out=out_r[:, tt, half * 512 : (half + 1) * 512], in_=rt[:]
                    )

        # ========== A + B interleaved with attention ==========
        with tc.tile_pool(name="pAB", bufs=1) as pAB:
            wqk_sb = pAB.tile([P, ND, D], BF16)
            nc.scalar.dma_start(out=wqk_sb[:], in_=wqk_r[:])
            wv_sb = pAB.tile([P, ND, HL * HD], BF16)
            nc.scalar.dma_start(out=wv_sb[:], in_=wv_r[:])

            def ab_chunk(qc):
                xlT = pAB.tile([P, ND, 512], BF16, tag="xlT", bufs=2)
                for t4 in range(4):
                    tt = 4 * qc + t4
                    x_tile = sc.tile([P, D], F32, tag="x_tile", bufs=2)
                    nc.sync.dma_start(out=x_tile[:], in_=x_r[:, tt, :])
                    xl = sc.tile([P, D], BF16, tag="xl")
                    layernorm_fold(sc, x_tile, xl)
                    for dt in range(ND):
                        tp = psMM.tile([P, P], BF16, tag="mm")
                        nc.tensor.transpose(
                            tp[:], xl[:, dt * P : (dt + 1) * P], ident[:]
                        )
                        nc.scalar.copy(
                            out=xlT[:, dt, t4 * P : (t4 + 1) * P], in_=tp[:]
                        )
                    vp = psMM.tile([P, 512], F32, tag="mm")
                    for dt in range(ND):
                        nc.tensor.matmul(
                            vp[:],
                            lhsT=xlT[:, dt, t4 * P : (t4 + 1) * P],
                            rhs=wv_sb[:, dt, :],
                            start=(dt == 0),
                            stop=(dt == ND - 1),
                        )
                    nc.vector.tensor_copy(
                        out=v_sb[:, tt, :, 0:HD],
                        in_=vp.rearrange("p (h e) -> p h e", h=HL),
                    )
                for ct in range(ND):
                    qp = psMM.tile([P, 512], F32, tag="mm")
                    for dt in range(ND):
                        nc.tensor.matmul(
                            qp[:],
                            lhsT=wqk_sb[:, dt, ct * P : (ct + 1) * P],
                            rhs=xlT[:, dt, :],
                            start=(dt == 0),
                            stop=(dt == ND - 1),
                        )
                    nc.scalar.activation(
                        out=qkT[:, ct, qc * 512 : (qc + 1) * 512],
                        in_=qp[:],
                        func=AF.Identity,
                        bias=qkb_sb[:, ct : ct + 1],
                        scale=1.0,
                    )

            ab_chunk(0)
            ab_chunk(1)
            attn(0)
            ab_chunk(2)
            attn(1, after_head={0: [lambda: post_d(0)]})
            ab_chunk(3)
            attn(2, after_head={0: [lambda: stage_e(0), lambda: post_d(1)]})

        # ---- F-phase SBUF on the RIGHT side (outlives attention PSUM pools) ----
        pG = es.enter_context(tc.tile_pool(name="pG", bufs=1, side="right"))
        sfw = es.enter_context(tc.tile_pool(name="sfw", bufs=4, side="right"))
        sfout = es.enter_context(tc.tile_pool(name="sfout", bufs=3, side="right"))
        gT = pG.tile([P, NFC, 512], BF16, tag="gT")
        pF_state["gT"] = gT

        units = [(0, fct) for fct in range(NFC)]
        uidx = [0]

        def emit_units(n):
            def cb():
                for _ in range(n):
                    if uidx[0] < len(units):
                        tc2, fct = units[uidx[0]]
                        uidx[0] += 1
                        fc1_unit(tc2, fct, psS, sfw)
            return cb

        sched = {
            0: [lambda: stage_e(1), lambda: post_d(2)],
            2: [lambda: stage_e(2)],
        }
        for hp, n in zip(range(1, 4), (11, 11, 10)):
            sched.setdefault(hp, []).append(emit_units(n))
        attn(3, after_head=sched)
        while uidx[0] < len(units):
            tc2, fct = units[uidx[0]]
            uidx[0] += 1
            fc1_unit(0, fct, psS, sfw)
        fc2_block(0, 0)
        emit_chunk_norm()
        post_d(3)
        fc2_block(0, 1)
        stage_e(3)
        for fct in range(NFC):
            fc1_unit(1, fct, psS, sfw)
        fc2_block(1, 0)
        fc2_block(1, 1)

    _split_sync_waits(nc)
    return nc


_NC_CACHE = None


def _get_nc():
    global _NC_CACHE
    if _NC_CACHE is None:
        _patch_tile_drain()
        _NC_CACHE = build_kernel()
    return _NC_CACHE


def make_in_maps(x, w_attn, w_o, ln1_g, ln1_b, ln2_g, ln2_b, w_fc, b_fc, w_proj, b_proj):
    bf = ml_dtypes.bfloat16
    f32 = np.float32
    x = np.asarray(x, f32)
    w_attn = np.asarray(w_attn, f32)
    w_o = np.asarray(w_o, f32)
    ln1_g = np.asarray(ln1_g, f32)
    ln1_b = np.asarray(ln1_b, f32)
    ln2_g = np.asarray(ln2_g, f32)
    ln2_b = np.asarray(ln2_b, f32)
    w_fc = np.asarray(w_fc, f32)
    b_fc = np.asarray(b_fc, f32)
    w_proj = np.asarray(w_proj, f32)
    b_proj = np.asarray(b_proj, f32)

    q_idx = np.arange(512)[None, :]
    k_idx = np.arange(P)[:, None]
    masks = np.stack(
        [(q_idx >= k_idx + P * j).astype(np.float32) for j in range(4)]
    )  # [4, P, 512]
    masks2 = masks.astype(bf)

    # sel[k, hp*128+m] = 1 iff head index (2hp + m//64) == k — broadcasts the
    # per-head reciprocal rows [8,512] to [128,512] per head-pair via matmul
    sel = np.zeros((HL, 512), np.float32)
    for hp in range(4):
        for m in range(P):
            sel[2 * hp + m // HD, hp * P + m] = 1.0
    sel = sel.astype(bf)

    wq = w_attn[:, :D]
    wk = w_attn[:, D : 2 * D]
    wv = w_attn[:, 2 * D :]

    wq_g = wq * ln1_g[:, None]
    wk_g = wk * ln1_g[:, None]
    wv_g = wv * ln1_g[:, None]
    wfc_g = w_fc * ln2_g[:, None]
    bfc_f = b_fc + ln2_b @ w_fc
    c0 = (ln1_b @ wv) @ w_o  # [D]

    in_maps = []
    for core in range(8):
        p, r = core // 2, core % 2
        hs = r * HL * HD
        wqk = np.concatenate([wq_g[:, hs : hs + 512], wk_g[:, hs : hs + 512]], axis=1)
        qkb = np.concatenate(
            [ln1_b @ wq[:, hs : hs + 512], ln1_b @ wk[:, hs : hs + 512]]
        )
        x_res = (
            np.concatenate(
                [x[p, 512 * c + 256 * r : 512 * c + 256 * r + 256] for c in range(4)],
                axis=0,
            )
            + c0[None, :]
        )
        in_maps.append(
            {
                "x": np.ascontiguousarray(x[p], f32),
                "x_res": np.ascontiguousarray(x_res, f32),
                "wqk": np.ascontiguousarray(wqk).astype(bf),
                "wv": np.ascontiguousarray(wv_g[:, hs : hs + 512]).astype(bf),
                "wo": np.ascontiguousarray(w_o[hs : hs + 512, :]).astype(bf),
                "wfc": np.ascontiguousarray(wfc_g).astype(bf),
                "wproj": np.ascontiguousarray(w_proj).astype(bf),
                "bfc": np.ascontiguousarray(bfc_f, f32),
                "bproj": np.ascontiguousarray(b_proj, f32),
                "qkb": np.ascontiguousarray(qkb, f32),
                "masks2": masks2,
                "sel": sel,
            }
        )
    return in_maps


def kernel(**inputs):
    inputs = {k: np.asarray(v) for k, v in inputs.items()}
    nc = _get_nc()
    in_maps = make_in_maps(**inputs)
    res = run_bass_kernel_spmd(nc, in_maps, core_ids=list(range(8)))
    x = inputs["x"]
    B = x.shape[0]
    out = np.empty((B, T, D), np.float32)
    for core in range(8):
        p, r = core // 2, core % 2
        o = res.results[core]["out"]
        for c in range(4):
            out[p, 512 * c + 256 * r : 512 * c + 256 * r + 256] = o[
                c * 256 : (c + 1) * 256
            ]
    return out


if __name__ == "__main__":
    print("building...")
    nc = _get_nc()
    n = sum(len(bb.instructions) for f in nc.m.functions for bb in f.blocks)
    print("built:", n, "instructions")


# revision 5
# speedup vs baseline: 1.1432x; 1.1124x over previous
"""Trainium2 Bass kernel v2 for the dense transformer block (B=4,T=2048,D=1024,H=16).

Same sharding as v1 (4 pairs x 2 ranks, head-split attention + 2-rank RS,
token-split MLP). Restructured for overlap:
  - LN gains/biases folded into weights host-side; LN = bn_stats + one ACT op.
  - A/B interleaved per token chunk; V matmuls before QK; attention chunk qc
    emitted right after its QK chunk.
  - Attention inner loop: score-MM pairs into 2-bank PSUM, ONE exp per
    [128,1024] pair, skewed one pair ahead of the AV matmuls.
  - Softmax denominators batched per chunk: one [8,512] DVE reciprocal,
    broadcast to head-pairs via a select-matrix matmul.
  - Post-chains (wo partial + RS + residual/LN2) emitted early, inside the
    next attention chunk.
  - MLP FC1 units interleaved into the ACT-bound attention phase; FC2 uses
    8 PSUM accumulators with wproj streamed (no 8MB resident tile).
"""

import sys
from contextlib import ExitStack

sys.path.insert(0, "/opt/trn_rl_repo")

import numpy as np
import ml_dtypes

import concourse.bass as bass
import concourse.tile as tile
from concourse import mybir
from concourse.bass_utils import run_bass_kernel_spmd
from concourse.masks import make_identity

BF16 = mybir.dt.bfloat16
F32 = mybir.dt.float32
AF = mybir.ActivationFunctionType

T = 2048
D = 1024
H = 16
HD = 64
HL = 8
P = 128
NT = T // P  # 16
ND = D // P  # 8
TL = T // 2  # 1024
NTL = TL // P  # 8
FC = 4 * D
NFC = FC // P  # 32
EPS = 1e-5


def _patch_tile_drain():
    from concourse.tile import ScopedClock

    def patched(self, tick_clock, wait_clock):
        nc = self.nc
        probe = nc.sync.nop(nofuse=True)
        wait_clock.add_sem_waits(probe.ins, ScopedClock({None: tick_clock.global_clock}))
        si = probe.ins.sync_info
        waits = list(si.on_wait) if si and si.on_wait else []
        if len(waits) > 1:
            probe.ins.sync_info = mybir.SyncInfo(
                on_wait=waits[:1], on_update=list(si.on_update or [])
            )
            for i in range(1, len(waits)):
                nop = nc.sync.nop(nofuse=True)
                nop.ins.sync_info = mybir.SyncInfo(on_wait=waits[i : i + 1], on_update=[])
        nc.all_engine_barrier()
        popped = nc._tile_sem_poison_stack.pop()
        assert popped is self._sem_poison
        nc.clear_and_free_semaphores(list(self.sems.allocated().values()))
        nc.all_engine_barrier()

    tile.TileContext._drain_and_barrier = patched


_SYNC_WAIT_CAP = 1


def _split_sync_waits(nc, cap=_SYNC_WAIT_CAP):
    cnt = 0
    for f in nc.m.functions:
        for bb in f.blocks:
            new_insts = []
            for inst in bb.instructions:
                si = inst.sync_info
                waits = list(si.on_wait) if si and si.on_wait else []
                if len(waits) > cap:
                    for i in range(0, len(waits) - cap, cap):
                        nop = mybir.InstNoOp(name=f"waitsplit_{cnt}", ins=[], outs=[])
                        cnt += 1
                        nop.engine = inst.engine
                        nop.sync_info = mybir.SyncInfo(
                            on_wait=waits[i : i + cap], on_update=[]
                        )
                        new_insts.append(nop)
                    inst.sync_info = mybir.SyncInfo(
                        on_wait=waits[len(waits) - cap :],
                        on_update=list(si.on_update or []),
                    )
                new_insts.append(inst)
            bb.instructions[:] = new_insts


def _bcast_ap(ap_1d, parts):
    return bass.AP(
        tensor=ap_1d.tensor,
        offset=ap_1d.offset,
        ap=[[0, parts], list(ap_1d.ap[0])],
    )


def build_kernel():
    nc = bass.Bass()

    x_ext = nc.declare_dram_parameter("x", [T, D], F32, isOutput=False)
    xres_ext = nc.declare_dram_parameter("x_res", [TL, D], F32, isOutput=False)
    wqk_ext = nc.declare_dram_parameter("wqk", [D, D], BF16, isOutput=False)
    wv_ext = nc.declare_dram_parameter("wv", [D, HL * HD], BF16, isOutput=False)
    wo_ext = nc.declare_dram_parameter("wo", [HL * HD, D], BF16, isOutput=False)
    wfc_ext = nc.declare_dram_parameter("wfc", [D, FC], BF16, isOutput=False)
    wproj_ext = nc.declare_dram_parameter("wproj", [FC, D], BF16, isOutput=False)
    bfc_ext = nc.declare_dram_parameter("bfc", [FC], F32, isOutput=False)
    bproj_ext = nc.declare_dram_parameter("bproj", [D], F32, isOutput=False)
    qkb_ext = nc.declare_dram_parameter("qkb", [D], F32, isOutput=False)
    masks2_ext = nc.declare_dram_parameter("masks2", [4, P, 512], BF16, isOutput=False)
    sel_ext = nc.declare_dram_parameter("sel", [HL, 512], BF16, isOutput=False)
    out_ext = nc.declare_dram_parameter("out", [TL, D], F32, isOutput=True)

    cc_ins = [nc.dram_tensor(f"cc_in{c}", [512, D], BF16) for c in range(4)]
    cc_outs = [nc.dram_tensor(f"cc_out{c}", [256, D], BF16) for c in range(4)]
    resid_dram = nc.dram_tensor("resid_dram", [TL, D], F32)

    x_r = x_ext.rearrange("(t p) d -> p t d", p=P)
    xres_r = xres_ext.rearrange("(t p) d -> p t d", p=P)
    wqk_r = wqk_ext.rearrange("(dt p) c -> p dt c", p=P)
    wv_r = wv_ext.rearrange("(dt p) c -> p dt c", p=P)
    wo_r = wo_ext.rearrange("(yt p) c -> p yt c", p=P)
    wfc_r = wfc_ext.rearrange("(dt p) c -> p dt c", p=P)
    wproj_r = wproj_ext.rearrange("(f p) d -> p f d", p=P)
    masks2_r = masks2_ext.rearrange("j p q -> p j q")
    qkb_r = qkb_ext.rearrange("(t p) -> p t", p=P)
    cc_in_rs = [t.rearrange("(t p) d -> p t d", p=P) for t in cc_ins]
    cc_out_rs = [t.rearrange("(t p) d -> p t d", p=P) for t in cc_outs]
    resid_r = resid_dram.rearrange("(t p) d -> p t d", p=P)
    out_r = out_ext.rearrange("(t p) d -> p t d", p=P)

    with tile.TileContext(nc) as tc, ExitStack() as es:
        singles = es.enter_context(tc.tile_pool(name="singles", bufs=1))
        pBC = es.enter_context(tc.tile_pool(name="pBC", bufs=1))
        sc = es.enter_context(tc.tile_pool(name="sc", bufs=3))
        scE = es.enter_context(tc.tile_pool(name="scE", bufs=2))

        # ---- constants ----
        ident = singles.tile([P, P], BF16)
        make_identity(nc, ident)
        bproj_sb = singles.tile([P, D], F32)
        nc.scalar.dma_start(out=bproj_sb[:], in_=_bcast_ap(bproj_ext[:], P))
        bfc_sb = singles.tile([P, NFC], F32)
        nc.scalar.dma_start(out=bfc_sb[:], in_=bfc_ext.rearrange("(o p) -> p o", p=P))
        masks2_sb = singles.tile([P, 4, 512], BF16)
        nc.scalar.dma_start(out=masks2_sb[:], in_=masks2_r[:])
        qkb_sb = singles.tile([P, ND], F32)
        nc.scalar.dma_start(out=qkb_sb[:], in_=qkb_r[:])
        sel_sb = singles.tile([HL, 4, P], BF16)
        nc.scalar.dma_start(out=sel_sb[:], in_=sel_ext.rearrange("k (hp m) -> k hp m", m=P))
        hT = singles.tile([P, ND, TL], BF16)  # LN2(resid)^T

        # ---- persistent SBUF for attention ----
        qkT = pBC.tile([P, ND, T], BF16)  # [qcols|kcols, tok]
        v_sb = pBC.tile([P, NT, HL, HD + 1], BF16)
        nc.vector.memset(v_sb[:, :, :, HD : HD + 1], 1.0)
        yT = pBC.tile([P, HL * HD // P, T], BF16)  # [ydim, tok]
        wo_sb = pBC.tile([P, HL * HD // P, D], BF16)
        nc.scalar.dma_start(out=wo_sb[:], in_=wo_r[:])

        def layernorm_fold(pool, x_tile, out_bf):
            """out_bf = (x - mu) * rstd  (gain/bias folded into weights)."""
            stats = pool.tile([P, 2, 6], F32, tag="ln_stats")
            for s in range(2):
                nc.vector.bn_stats(
                    out=stats[:, s, :], in_=x_tile[:, s * 512 : (s + 1) * 512]
                )
            mv = pool.tile([P, 2], F32, tag="ln_mv")
            nc.vector.bn_aggr(out=mv[:], in_=stats[:])
            # rstd = 1/sqrt(var+eps) via Newton on DVE (keeps ACT tables
            # free for Exp/Gelu; LN input variance is ~1 so 3 steps converge)
            ve = pool.tile([P, 1], F32, tag="ln_ve")
            nc.vector.tensor_scalar(
                out=ve[:], in0=mv[:, 1:2], scalar1=EPS, scalar2=None,
                op0=mybir.AluOpType.add,
            )
            y = pool.tile([P, 1], F32, tag="ln_y")
            nc.vector.tensor_scalar(
                out=y[:], in0=ve[:], scalar1=-0.5, scalar2=1.5,
                op0=mybir.AluOpType.mult, op1=mybir.AluOpType.add,
            )
            w = pool.tile([P, 1], F32, tag="ln_w")
            for _ in range(2):
                nc.vector.tensor_mul(out=w[:], in0=y[:], in1=y[:])
                nc.vector.tensor_scalar(
                    out=w[:], in0=w[:], scalar1=ve[:], scalar2=-0.5,
                    op0=mybir.AluOpType.mult, op1=mybir.AluOpType.mult,
                )
                nc.vector.tensor_scalar(
                    out=w[:], in0=w[:], scalar1=1.5, scalar2=None,
                    op0=mybir.AluOpType.add,
                )
                nc.vector.tensor_mul(out=y[:], in0=y[:], in1=w[:])
            nmr = pool.tile([P, 1], F32, tag="ln_nmr")
            nc.vector.tensor_scalar(
                out=nmr[:],
                in0=mv[:, 0:1],
                scalar1=y[:],
                scalar2=-1.0,
                op0=mybir.AluOpType.mult,
                op1=mybir.AluOpType.mult,
            )
            nc.scalar.activation(
                out=out_bf[:], in_=x_tile[:], func=AF.Identity,
                bias=nmr[:], scale=y[:],
            )

        psMM = es.enter_context(tc.tile_pool(name="psMM", bufs=2, space="PSUM"))
        psS = es.enter_context(tc.tile_pool(name="psS", bufs=2, space="PSUM"))
        psY = es.enter_context(tc.tile_pool(name="psY", bufs=2, space="PSUM"))

        # ========== stage E (inside post_chain) ==========
        def stage_e(pqc):
            for i2 in range(2):
                tt = 2 * pqc + i2
                rs_bf = scE.tile([P, D], BF16, tag="rs_bf")
                nc.gpsimd.dma_start(out=rs_bf[:], in_=cc_out_rs[pqc][:, i2, :])
                xr = scE.tile([P, D], F32, tag="xr")
                nc.gpsimd.dma_start(out=xr[:], in_=xres_r[:, tt, :])
                nc.vector.tensor_add(out=xr[:], in0=xr[:], in1=rs_bf[:])
                nc.gpsimd.dma_start(out=resid_r[:, tt, :], in_=xr[:])
                h2 = sc.tile([P, D], BF16, tag="xl")
                layernorm_fold(scE, xr, h2)
                for dt in range(ND):
                    tp2 = psMM.tile([P, P], BF16, tag="mm")
                    nc.tensor.transpose(tp2[:], h2[:, dt * P : (dt + 1) * P], ident[:])
                    nc.vector.tensor_copy(
                        out=hT[:, dt, tt * P : (tt + 1) * P], in_=tp2[:]
                    )

        # ========== post chain: D (wo partial) + RS ==========
        def post_d(pqc):
            for t4 in range(4):
                tt = 4 * pqc + t4
                for half in range(2):
                    op = psMM.tile([P, 512], F32, tag="mm")
                    for yt in range(HL * HD // P):
                        nc.tensor.matmul(
                            op[:],
                            lhsT=yT[:, yt, tt * P : (tt + 1) * P],
                            rhs=wo_sb[:, yt, half * 512 : (half + 1) * 512],
                            start=(yt == 0),
                            stop=(yt == HL * HD // P - 1),
                        )
                    ob = sc.tile([P, 512], BF16, tag="ob", bufs=2)
                    nc.vector.tensor_copy(out=ob[:], in_=op[:])
                    nc.gpsimd.dma_start(
                        out=cc_in_rs[pqc][:, t4, half * 512 : (half + 1) * 512],
                        in_=ob[:],
                    )
            nc.gpsimd.collective_compute(
                "ReduceScatter",
                mybir.AluOpType.add,
                ins=[cc_ins[pqc][:]],
                outs=[cc_outs[pqc][:]],
                replica_groups=[[0, 1], [2, 3], [4, 5], [6, 7]],
            )

        # ========== attention chunk ==========
        norm_state = {"pending": None}

        def emit_chunk_norm():
            if norm_state["pending"] is None:
                return
            qc, ys2_list, dsum8 = norm_state["pending"]
            norm_state["pending"] = None
            nc.vector.reciprocal(out=dsum8[:], in_=dsum8[:])
            recb8 = sc.tile([HL, 512], BF16, tag="recb8", bufs=2)
            nc.vector.tensor_copy(out=recb8[:], in_=dsum8[:])
            for hp in range(4):
                bp2 = psMM.tile([P, 512], F32, tag="mm")
                nc.tensor.matmul(
                    bp2[:], lhsT=sel_sb[:, hp, :], rhs=recb8[:], start=True, stop=True
                )
                bps2 = sc.tile([P, 512], BF16, tag="bps2", bufs=2)
                nc.vector.tensor_copy(out=bps2[:], in_=bp2[:])
                nc.vector.tensor_mul(
                    out=yT[:, hp, qc * 512 : (qc + 1) * 512],
                    in0=ys2_list[hp][:],
                    in1=bps2[:],
                )

        def attn(qc, after_head=None):
            nkt = 4 * (qc + 1)
            dsum8 = sc.tile([HL, 512], F32, tag="dsum8", bufs=2, name=f"dsum8_q{qc}")
            ys2_list = []
            for hp in range(4):
                hA, hB = 2 * hp, 2 * hp + 1
                qt_ct = hp
                kt_ct = 4 + hp
                ys2 = sc.tile([P, 512], BF16, tag="ys2", bufs=6, name=f"ys2_q{qc}p{hp}")
                ys2_list.append(ys2)
                ypA = psY.tile([HD + 1, 512], F32, tag="yp", name=f"ypA_q{qc}p{hp}")
                ypB = psY.tile([HD + 1, 512], F32, tag="yp", name=f"ypB_q{qc}p{hp}")
                pending_av = [None]

                def emit_av(ypA=ypA, ypB=ypB, pending_av=pending_av, nkt=nkt, hA=hA, hB=hB):
                    if pending_av[0] is None:
                        return
                    pt2, kt = pending_av[0]
                    pending_av[0] = None
                    nc.tensor.matmul(
                        ypA[:], lhsT=v_sb[:, kt, hA, :], rhs=pt2[:, 0:512],
                        start=(kt == 0), stop=(kt == nkt - 1),
                    )
                    nc.tensor.matmul(
                        ypB[:], lhsT=v_sb[:, kt, hB, :], rhs=pt2[:, 512:1024],
                        start=(kt == 0), stop=(kt == nkt - 1),
                    )

                for kt in range(nkt):
                    ps2 = psS.tile([P, 1024], F32, tag="s2", name=f"s_q{qc}p{hp}k{kt}")
                    # two heads' score MMs run concurrently in disjoint row
                    # strips (K=64 each, bases 0 and 64) into separate banks
                    nc.tensor.matmul(
                        ps2[:, 0:512],
                        lhsT=qkT[0:HD, kt_ct, kt * P : (kt + 1) * P],
                        rhs=qkT[0:HD, qt_ct, qc * 512 : (qc + 1) * 512],
                        start=True, stop=True,
                    )
                    nc.tensor.matmul(
                        ps2[:, 512:1024],
                        lhsT=qkT[HD:P, kt_ct, kt * P : (kt + 1) * P],
                        rhs=qkT[HD:P, qt_ct, qc * 512 : (qc + 1) * 512],
                        start=True, stop=True,
                    )
                    if kt == 0 and hp == 0:
                        emit_chunk_norm()  # previous chunk's normalize
                    pt2 = sc.tile([P, 1024], BF16, tag="pt2", name=f"pt_q{qc}p{hp}k{kt}")
                    nc.scalar.activation(
                        out=pt2[:], in_=ps2[:], func=AF.Exp, scale=0.125
                    )
                    j = kt - 4 * qc
                    if j >= 0:
                        for hh in range(2):
                            nc.vector.tensor_mul(
                                out=pt2[:, hh * 512 : (hh + 1) * 512],
                                in0=pt2[:, hh * 512 : (hh + 1) * 512],
                                in1=masks2_sb[:, j, :],
                            )
                    emit_av()
                    pending_av[0] = (pt2, kt)
                emit_av()
                nc.vector.tensor_copy(out=ys2[0:HD, :], in_=ypA[0:HD, :])
                nc.vector.tensor_copy(out=ys2[HD:P, :], in_=ypB[0:HD, :])
                for h, yp in ((hA, ypA), (hB, ypB)):
                    dcp = sc.tile([1, 512], F32, tag="dcp", name=f"dcp_q{qc}h{h}")
                    nc.vector.tensor_copy(out=dcp[:], in_=yp[HD : HD + 1, :])
                    nc.gpsimd.dma_start(out=dsum8[h : h + 1, :], in_=dcp[:])
                if after_head is not None:
                    for cb in after_head.get(hp, []):
                        cb()
            norm_state["pending"] = (qc, ys2_list, dsum8)

        # ========== MLP pieces ==========
        pF_state = {"fc1tag": "s2"}

        def fc1_unit(tc2, fct, psum_pool, sfpool):
            wfc_tile = sfpool.tile([P, ND, P], BF16, tag="wfc_t", bufs=4)
            nc.sync.dma_start(
                out=wfc_tile[:], in_=wfc_r[:, :, fct * P : (fct + 1) * P]
            )
            fp = psum_pool.tile([P, 512], F32, tag=pF_state["fc1tag"])
            for dt in range(ND):
                nc.tensor.matmul(
                    fp[:],
                    lhsT=wfc_tile[:, dt, :],
                    rhs=hT[:, dt, tc2 * 512 : (tc2 + 1) * 512],
                    start=(dt == 0),
                    stop=(dt == ND - 1),
                )
            gT = pF_state["gT"]
            nc.scalar.activation(
                out=gT[:, fct, :],
                in_=fp[:],
                func=AF.Gelu_apprx_tanh,
                bias=bfc_sb[:, fct : fct + 1],
                scale=1.0,
            )

        def fc2_block(tc2, t4pair):
            gT = pF_state["gT"]
            accs = [
                psS.tile([P, 1024], F32, tag="s2", name=f"facc{tc2}_{t4pair}_{i}")
                for i in range(2)
            ]
            for fct in range(NFC):
                wp = sfw.tile(
                    [P, D], BF16, tag="wp_t", bufs=4, name=f"wp{tc2}_{t4pair}_{fct}"
                )
                nc.sync.dma_start(out=wp[:], in_=wproj_r[:, fct, :])
                for i2 in range(2):
                    t4 = 2 * t4pair + i2
                    for half in range(2):
                        nc.tensor.matmul(
                            accs[i2][:, half * 512 : (half + 1) * 512],
                            lhsT=gT[:, fct, t4 * P : (t4 + 1) * P],
                            rhs=wp[:, half * 512 : (half + 1) * 512],
                            start=(fct == 0),
                            stop=(fct == NFC - 1),
                        )
            for i2 in range(2):
                t4 = 2 * t4pair + i2
                tt = tc2 * 4 + t4
                for half in range(2):
                    rt = sfout.tile([P, 512], F32, tag="rt")
                    nc.sync.dma_start(
                        out=rt[:], in_=resid_r[:, tt, half * 512 : (half + 1) * 512]
                    )
                    nc.vector.tensor_add(
                        out=rt[:],
                        in0=accs[i2][:, half * 512 : (half + 1) * 512],
                        in1=rt[:],
                    )
                    nc.vector.tensor_add(
                        out=rt[:],
                        in0=rt[:],
                        in1=bproj_sb[:, half * 512 : (half + 1) * 512],
                    )
                    nc.sync.dma_start(
                        out=out_r[:, tt, half * 512 : (half + 1) * 512], in_=rt[:]
                    )

        # ========== A + B interleaved with attention ==========
        with tc.tile_pool(name="pAB", bufs=1) as pAB:
            wqk_sb = pAB.tile([P, ND, D], BF16)
            nc.scalar.dma_start(out=wqk_sb[:], in_=wqk_r[:])
            wv_sb = pAB.tile([P, ND, HL * HD], BF16)
            nc.scalar.dma_start(out=wv_sb[:], in_=wv_r[:])

            def ab_chunk(qc):
                xlT = pAB.tile([P, ND, 512], BF16, tag="xlT", bufs=2)
                for t4 in range(4):
                    tt = 4 * qc + t4
                    x_tile = sc.tile([P, D], F32, tag="x_tile", bufs=2)
                    nc.sync.dma_start(out=x_tile[:], in_=x_r[:, tt, :])
                    xl = sc.tile([P, D], BF16, tag="xl")
                    layernorm_fold(sc, x_tile, xl)
                    for dt in range(ND):
                        tp = psMM.tile([P, P], BF16, tag="mm")
                        nc.tensor.transpose(
                            tp[:], xl[:, dt * P : (dt + 1) * P], ident[:]
                        )
                        nc.scalar.copy(
                            out=xlT[:, dt, t4 * P : (t4 + 1) * P], in_=tp[:]
                        )
                    vp = psMM.tile([P, 512], F32, tag="mm")
                    for dt in range(ND):
                        nc.tensor.matmul(
                            vp[:],
                            lhsT=xlT[:, dt, t4 * P : (t4 + 1) * P],
                            rhs=wv_sb[:, dt, :],
                            start=(dt == 0),
                            stop=(dt == ND - 1),
                        )
                    nc.vector.tensor_copy(
                        out=v_sb[:, tt, :, 0:HD],
                        in_=vp.rearrange("p (h e) -> p h e", h=HL),
                    )
                for ct in range(ND):
                    qp = psMM.tile([P, 512], F32, tag="mm")
                    for dt in range(ND):
                        nc.tensor.matmul(
                            qp[:],
                            lhsT=wqk_sb[:, dt, ct * P : (ct + 1) * P],
                            rhs=xlT[:, dt, :],
                            start=(dt == 0),
                            stop=(dt == ND - 1),
                        )
                    nc.vector.tensor_scalar(
                        out=qkT[:, ct, qc * 512 : (qc + 1) * 512],
                        in0=qp[:],
                        scalar1=qkb_sb[:, ct : ct + 1],
                        scalar2=None,
                        op0=mybir.AluOpType.add,
                    )

            ab_chunk(0)
            ab_chunk(1)
            attn(0)
            ab_chunk(2)
            attn(1, after_head={0: [lambda: post_d(0)]})
            ab_chunk(3)
            attn(2, after_head={0: [lambda: stage_e(0), lambda: post_d(1)], 2: [lambda: stage_e(1)]})

        # ---- F-phase SBUF on the RIGHT side (outlives attention PSUM pools) ----
        pG = es.enter_context(tc.tile_pool(name="pG", bufs=1, side="right"))
        sfw = es.enter_context(tc.tile_pool(name="sfw", bufs=4, side="right"))
        sfout = es.enter_context(tc.tile_pool(name="sfout", bufs=3, side="right"))
        gT = pG.tile([P, NFC, 512], BF16, tag="gT")
        pF_state["gT"] = gT

        units = [(0, fct) for fct in range(NFC)]
        uidx = [0]

        def emit_units(n):
            def cb():
                for _ in range(n):
                    if uidx[0] < len(units):
                        tc2, fct = units[uidx[0]]
                        uidx[0] += 1
                        fc1_unit(tc2, fct, psS, sfw)
            return cb

        sched = {
            0: [lambda: post_d(2)],
            2: [lambda: stage_e(2)],
        }
        for hp, n in zip(range(0, 4), (8, 8, 8, 8)):
            sched.setdefault(hp, []).append(emit_units(n))
        attn(3, after_head=sched)
        while uidx[0] < len(units):
            tc2, fct = units[uidx[0]]
            uidx[0] += 1
            fc1_unit(0, fct, psS, sfw)
        fc2_block(0, 0)
        emit_chunk_norm()
        post_d(3)
        fc2_block(0, 1)
        stage_e(3)
        for fct in range(NFC):
            fc1_unit(1, fct, psS, sfw)
        fc2_block(1, 0)
        fc2_block(1, 1)

    _split_sync_waits(nc)
    return nc


_NC_CACHE = None


def _get_nc():
    global _NC_CACHE
    if _NC_CACHE is None:
        _patch_tile_drain()
        _NC_CACHE = build_kernel()
    return _NC_CACHE


def make_in_maps(x, w_attn, w_o, ln1_g, ln1_b, ln2_g, ln2_b, w_fc, b_fc, w_proj, b_proj):
    bf = ml_dtypes.bfloat16
    f32 = np.float32
    x = np.asarray(x, f32)
    w_attn = np.asarray(w_attn, f32)
    w_o = np.asarray(w_o, f32)
    ln1_g = np.asarray(ln1_g, f32)
    ln1_b = np.asarray(ln1_b, f32)
    ln2_g = np.asarray(ln2_g, f32)
    ln2_b = np.asarray(ln2_b, f32)
    w_fc = np.asarray(w_fc, f32)
    b_fc = np.asarray(b_fc, f32)
    w_proj = np.asarray(w_proj, f32)
    b_proj = np.asarray(b_proj, f32)

    q_idx = np.arange(512)[None, :]
    k_idx = np.arange(P)[:, None]
    masks = np.stack(
        [(q_idx >= k_idx + P * j).astype(np.float32) for j in range(4)]
    )  # [4, P, 512]
    masks2 = masks.astype(bf)

    # sel[k, hp*128+m] = 1 iff head index (2hp + m//64) == k — broadcasts the
    # per-head reciprocal rows [8,512] to [128,512] per head-pair via matmul
    sel = np.zeros((HL, 512), np.float32)
    for hp in range(4):
        for m in range(P):
            sel[2 * hp + m // HD, hp * P + m] = 1.0
    sel = sel.astype(bf)

    wq = w_attn[:, :D]
    wk = w_attn[:, D : 2 * D]
    wv = w_attn[:, 2 * D :]

    wq_g = wq * ln1_g[:, None]
    wk_g = wk * ln1_g[:, None]
    wv_g = wv * ln1_g[:, None]
    wfc_g = w_fc * ln2_g[:, None]
    bfc_f = b_fc + ln2_b @ w_fc
    c0 = (ln1_b @ wv) @ w_o  # [D]

    in_maps = []
    for core in range(8):
        p, r = core // 2, core % 2
        hs = r * HL * HD
        wqk = np.concatenate([wq_g[:, hs : hs + 512], wk_g[:, hs : hs + 512]], axis=1)
        qkb = np.concatenate(
            [ln1_b @ wq[:, hs : hs + 512], ln1_b @ wk[:, hs : hs + 512]]
        )
        x_res = (
            np.concatenate(
                [x[p, 512 * c + 256 * r : 512 * c + 256 * r + 256] for c in range(4)],
                axis=0,
            )
            + c0[None, :]
        )
        in_maps.append(
            {
                "x": np.ascontiguousarray(x[p], f32),
                "x_res": np.ascontiguousarray(x_res, f32),
                "wqk": np.ascontiguousarray(wqk).astype(bf),
                "wv": np.ascontiguousarray(wv_g[:, hs : hs + 512]).astype(bf),
                "wo": np.ascontiguousarray(w_o[hs : hs + 512, :]).astype(bf),
                "wfc": np.ascontiguousarray(wfc_g).astype(bf),
                "wproj": np.ascontiguousarray(w_proj).astype(bf),
                "bfc": np.ascontiguousarray(bfc_f, f32),
                "bproj": np.ascontiguousarray(b_proj, f32),
                "qkb": np.ascontiguousarray(qkb, f32),
                "masks2": masks2,
                "sel": sel,
            }
        )
    return in_maps


def kernel(**inputs):
    inputs = {k: np.asarray(v) for k, v in inputs.items()}
    nc = _get_nc()
    in_maps = make_in_maps(**inputs)
    res = run_bass_kernel_spmd(nc, in_maps, core_ids=list(range(8)))
    x = inputs["x"]
    B = x.shape[0]
    out = np.empty((B, T, D), np.float32)
    for core in range(8):
        p, r = core // 2, core % 2
        o = res.results[core]["out"]
        for c in range(4):
            out[p, 512 * c + 256 * r : 512 * c + 256 * r + 256] = o[
                c * 256 : (c + 1) * 256
            ]
    return out


if __name__ == "__main__":
    print("building...")
    nc = _get_nc()
    n = sum(len(bb.instructions) for f in nc.m.functions for bb in f.blocks)
    print("built:", n, "instructions")
